# revision 28
# baseline (speedup 1.0000x reference)
"""Causal single-head attention (B=8, S=2048, D=1024, fp32) on 8 NeuronCores.

Data-parallel over batch: one batch element per core, weights replicated.
Fully SBUF-resident (no DRAM scratch), fp32 PSUM accumulation.

All GEMM phases (V/Q/K projections and scores) run in two-component fp8
e4m3 using DoubleRow perf mode (2 k-tiles per instruction, 0.5 cycles/row).
Each operand T is pre-scaled by 16 (to sit in e4m3's normal range) and split
as 16T ~= Th + Tl with Tl = e4m3(16T - Th). A product then needs 3 DoubleRow
passes (Th*Uh + Th*Ul + Tl*Uh, dropping the ~eps^2 Tl*Ul term) accumulated
in one PSUM chain = 0.75 cycles/row total, 25% faster than bf16 at bf16-level
accuracy (quantization error ~0.06% vs bf16's 0.4%). The 16*16 scale factors
are divided back out on the PSUM->SBUF copies (or folded into exp's scale).

Host pre-casts inputs (x -> bf16; 16*W -> e4m3 hi/lo pairs): 10MB in / 4MB
out per core, no on-device weight casts.

Per-core pipeline:
  1. xT = x.T via PE transposes (bf16, 1 cycle/row); PSUM copied to
     xt8h = e4m3(16 xT) (ScalarE) and xt8l = e4m3(16 xT - xt8h) (VectorE)
  2. V  = x @ Wv    -> vsb [ki, kt, e] bf16 (scaled 1/256 on copy)
     QT = Wq.T @ xT -> qt8h/qt8l [e, et, s] e4m3 pair (scaled 1/16 on copy)
     KT = Wk.T @ xT -> kt8h/kt8l [e, et, s] e4m3 pair
  3. per 256-wide query chunk c2, k-tile k<=2*c2+1: S^T accumulated over
     4 e-tile-pairs x 3 hi/lo passes; exp(s/8192) on ScalarE -> est bf16;
     diagonal tiles multiplied by a 0/1 causal mask on GpSimd
  4. per 128-row q tile: rowsum + AV in bf16 via est-weight-reuse matmul
     groups (rs N=2 self-loading, then 2x N=512 with ldweights=False),
     normalize on VectorE (bf16 out), DMA out. Host widens to fp32.
"""

import numpy as np

B, S, D = 8, 2048, 1024
P = 128
NCORES = 8

_built = None


def _bf16_bits(a):
    """fp32 ndarray -> uint16 bf16 bits, round-to-nearest-even."""
    b = np.ascontiguousarray(a, dtype=np.float32).view(np.uint32)
    r = b + 0x7FFF + ((b >> 16) & 1)
    return (r >> 16).astype(np.uint16)


def _e4m3_hi_lo(a, scale=16.0):
    """fp32 ndarray -> (hi, lo) uint8 e4m3fn bit arrays with
    hi = e4m3(scale*a), lo = e4m3(scale*a - hi)."""
    import ml_dtypes

    x = np.ascontiguousarray(a, dtype=np.float32) * np.float32(scale)
    np.clip(x, -448.0, 448.0, out=x)
    hi = x.astype(ml_dtypes.float8_e4m3fn)
    lo = (x - hi.astype(np.float32)).astype(ml_dtypes.float8_e4m3fn)
    return hi.view(np.uint8), lo.view(np.uint8)


def _build():
    import concourse.tile as tile
    import concourse.mybir as mybir
    from concourse import bacc

    FP32 = mybir.dt.float32
    BF16 = mybir.dt.bfloat16
    FP8 = mybir.dt.float8e4
    U16 = mybir.dt.uint16
    U8 = mybir.dt.uint8
    AF = mybir.ActivationFunctionType
    DR = mybir.MatmulPerfMode.DoubleRow
    MUL = mybir.AluOpType.mult
    SUB = mybir.AluOpType.subtract

    nc = bacc.Bacc("TRN2", target_bir_lowering=False, debug=False, num_devices=NCORES)
    x_d = nc.dram_tensor("x16", [S, D], U16, kind="ExternalInput").ap().bitcast(BF16)
    w8_d = {}
    for wn in ("Wq8h", "Wq8l", "Wk8h", "Wk8l", "Wv8h", "Wv8l"):
        w8_d[wn] = (
            nc.dram_tensor(wn, [D, D], U8, kind="ExternalInput").ap().bitcast(FP8)
        )
    out_d = nc.dram_tensor("out", [S, D], U16, kind="ExternalOutput").ap().bitcast(BF16)

    ident_c = nc.inline_tensor(
        (np.eye(P) * 0x3F80).astype(np.uint16), name="ident_c"
    )
    # sliding causal 0/1 mask in bf16 bits: 256-wide tile j2 uses cols
    # [(3-j2)*128, +256); value at (p, y) = 1 iff y >= p + 384 else 0.
    # cols 896:898 = ones (rowsum rhs).
    zz = np.arange(896)[None, :]
    pp = np.arange(P)[:, None]
    m01 = np.where(zz >= pp + 384, 0x3F80, 0x0000).astype(np.uint16)
    m01 = np.concatenate([m01, np.full((P, 2), 0x3F80, np.uint16)], axis=1)
    mask_c = nc.inline_tensor(m01, name="mask_c")

    with tile.TileContext(nc) as tc:
        with (
            tc.tile_pool(name="x8p", bufs=1) as x8p,
            tc.tile_pool(name="qtp", bufs=1) as qtp,
            tc.tile_pool(name="ktp", bufs=1) as ktp,
            tc.tile_pool(name="vbp", bufs=1) as vbp,
            tc.tile_pool(name="xsp", bufs=6) as xsp,
            tc.tile_pool(name="w8p", bufs=4) as w8p,
            tc.tile_pool(name="estp", bufs=32) as estp,
            tc.tile_pool(name="osbp", bufs=2) as osbp,
            tc.tile_pool(name="smp", bufs=1) as smp,
            tc.tile_pool(name="rcpp", bufs=2) as rcpp,
            tc.tile_pool(name="ps", bufs=8, space="PSUM") as ps,
        ):
            mask = smp.tile([P, 898], BF16, tag="mask")
            nc.sync.dma_start(out=mask, in_=mask_c.ap().bitcast(BF16))
            ident = smp.tile([P, P], BF16, tag="ident")
            nc.sync.dma_start(out=ident, in_=ident_c.ap().bitcast(BF16))
            ones2 = mask[:, 896:898]
            # memset (no DMA dependency) operand for the p-state warmup
            # matmuls; results never read
            junk = smp.tile([P, 640], BF16, tag="junk")
            nc.vector.memset(junk, 0.0)

            xt8h = x8p.tile([P, 8, S], FP8, tag="xt8h")
            xt8l = x8p.tile([P, 8, S], FP8, tag="xt8l")
            qt8h = qtp.tile([P, 8, S], FP8, tag="qt8h")
            qt8l = qtp.tile([P, 8, S], FP8, tag="qt8l")
            kt8h = ktp.tile([P, 8, S], FP8, tag="kt8h")
            kt8l = ktp.tile([P, 8, S], FP8, tag="kt8l")
            vsb = vbp.tile([P, 16, D], BF16, tag="vsb")

            # PE p-state warmup (no DMA dependency)
            for r in range(14):
                dps = ps.tile([P, 512], FP32, tag="ps", name=f"warm{r}")
                nc.tensor.matmul(
                    dps, lhsT=junk[:, 0:P], rhs=junk[:, 128:640],
                    start=True, stop=True,
                )

            x_pend = {}

            def load_x(si):
                x_tile = xsp.tile([P, D], BF16, tag="xs", name=f"x{si}")
                eng = (nc.sync, nc.gpsimd)[si % 2]
                eng.dma_start(out=x_tile, in_=x_d[si * P:(si + 1) * P, :])
                x_pend[si] = x_tile

            def load_w8(wname):
                """One of Wq/Wk/Wv as an (hi, lo) pair of [P, 8, D] tiles."""
                pair = []
                for part in ("h", "l"):
                    w8 = w8p.tile([P, 8, D], FP8, tag="w8", name=f"{wname}{part}")
                    for kd in range(8):
                        eng = (nc.sync, nc.gpsimd)[kd % 2]
                        eng.dma_start(
                            out=w8[:, kd, :],
                            in_=w8_d[f"{wname}8{part}"][kd * P:(kd + 1) * P, :],
                        )
                    pair.append(w8)
                return pair

            # ---- phase A: transpose group g -> xt8h/xt8l columns ----
            def transpose_group(g):
                xts = [x_pend.pop(si) for si in range(4 * g, 4 * g + 4)]
                for kd in range(8):
                    tp4 = ps.tile([P, 512], BF16, tag="ps", name=f"tp{g}_{kd}")
                    for j in range(4):
                        nc.tensor.matmul(
                            tp4[:, j * P:(j + 1) * P],
                            lhsT=xts[j][:, kd * P:(kd + 1) * P],
                            rhs=ident,
                            is_transpose=True,
                            start=(j == 0),
                            stop=(j == 3),
                        )
                    hsl = xt8h[:, kd, g * 512:(g + 1) * 512]
                    nc.scalar.mul(hsl, tp4, 16.0)
                    nc.vector.scalar_tensor_tensor(
                        out=xt8l[:, kd, g * 512:(g + 1) * 512],
                        in0=tp4, scalar=16.0, in1=hsl, op0=MUL, op1=SUB,
                    )

            def dr_chain(pst, passes, lhs_of, rhs_of):
                """3 hi/lo DoubleRow passes x 4 k-tile pairs into one PSUM."""
                n = len(passes) * 4 - 1
                i = 0
                for lt, rt in passes:
                    for kd in range(0, 8, 2):
                        nc.tensor.matmul(
                            pst,
                            lhsT=lhs_of(lt, kd),
                            rhs=rhs_of(rt, kd),
                            start=(i == 0),
                            stop=(i == n),
                            perf_mode=DR,
                        )
                        i += 1

            ncopy = 0

            # ---- phase B: V projection for group g ----
            def v_proj_group(g, wvh, wvl):
                nonlocal ncopy
                for st_i in range(4 * g, 4 * g + 4):
                    for ec in range(2):
                        pst = ps.tile([P, 512], FP32, tag="ps", name=f"v{st_i}_{ec}")
                        dr_chain(
                            pst,
                            [(xt8h, wvh), (xt8h, wvl), (xt8l, wvh)],
                            lambda xs, kd: xs[:, kd:kd + 2, st_i * P:(st_i + 1) * P],
                            lambda w, kd: w[:, kd:kd + 2, ec * 512:(ec + 1) * 512],
                        )
                        dst = vsb[:, st_i, ec * 512:(ec + 1) * 512]
                        if ncopy % 2 == 0:
                            nc.vector.tensor_scalar_mul(dst, pst, 1.0 / 256.0)
                        else:
                            nc.scalar.mul(dst, pst, 1.0 / 256.0)
                        ncopy += 1

            # ---- phase C/D: Q^T / K^T projections -> fp8 hi/lo pairs ----
            def qk_proj_sc(sc, wh, wl, dsth, dstl):
                for et in range(8):
                    pst = ps.tile([P, 512], FP32, tag="ps", name=f"p{sc}_{et}")
                    dr_chain(
                        pst,
                        [(wh, xt8h), (wl, xt8h), (wh, xt8l)],
                        lambda w, kd: w[:, kd:kd + 2, et * P:(et + 1) * P],
                        lambda xs, kd: xs[:, kd:kd + 2, sc * 512:(sc + 1) * 512],
                    )
                    hsl = dsth[:, et, sc * 512:(sc + 1) * 512]
                    nc.scalar.mul(hsl, pst, 1.0 / 16.0)
                    nc.vector.scalar_tensor_tensor(
                        out=dstl[:, et, sc * 512:(sc + 1) * 512],
                        in0=pst, scalar=1.0 / 16.0, in1=hsl, op0=MUL, op1=SUB,
                    )

            # ---- phase E: S^T + exp for 256-wide chunk c2 ----
            est_tiles = {}

            def s_stage(c2):
                for k in range(2 * c2 + 2):
                    sps = ps.tile([P, 256], FP32, tag="ps", name=f"s{c2}_{k}")
                    dr_chain(
                        sps,
                        [(kt8h, qt8h), (kt8h, qt8l), (kt8l, qt8h)],
                        lambda kt, e: kt[:, e:e + 2, k * P:(k + 1) * P],
                        lambda qt, e: qt[:, e:e + 2, c2 * 256:(c2 + 1) * 256],
                    )
                    est = estp.tile([P, 256], BF16, tag="est", name=f"e{c2}_{k}")
                    # (16Q)(16K) = 256 S -> 1/(32*256) = 1/8192
                    nc.scalar.activation(
                        out=est, in_=sps, func=AF.Exp, scale=1.0 / 8192.0
                    )
                    j2 = k - 2 * c2
                    if j2 >= 0:
                        nc.gpsimd.tensor_mul(
                            est, est, mask[:, (3 - j2) * P:(3 - j2) * P + 256]
                        )
                    est_tiles[(c2, k)] = est

            # ---- phase F: rowsum + AV + normalize for 512-chunk c ----
            def av_stage(c):
                for j in range(4):
                    q_abs = 4 * c + j
                    rs = ps.tile([P, 2], FP32, tag="ps", name=f"rs{q_abs}")
                    o0 = ps.tile([P, 512], FP32, tag="ps", name=f"o0_{q_abs}")
                    o1 = ps.tile([P, 512], FP32, tag="ps", name=f"o1_{q_abs}")
                    c2 = 2 * c + j // 2
                    j2 = j % 2
                    for k in range(q_abs + 1):
                        lhs = est_tiles[(c2, k)][:, j2 * P:(j2 + 1) * P]
                        st = (k == 0)
                        sp = (k == q_abs)
                        nc.tensor.matmul(rs, lhsT=lhs, rhs=ones2, start=st, stop=sp)
                        m2 = nc.tensor.matmul(
                            o0, lhsT=lhs, rhs=vsb[:, k, 0:512], start=st, stop=sp
                        )
                        m2.ins.ldweights = False
                        m3 = nc.tensor.matmul(
                            o1, lhsT=lhs, rhs=vsb[:, k, 512:1024], start=st, stop=sp
                        )
                        m3.ins.ldweights = False
                    rec = rcpp.tile([P, 1], FP32, tag="rcp", name=f"rc{q_abs}")
                    nc.vector.reciprocal(rec, rs[:, 0:1])
                    o_sb = osbp.tile([P, D], BF16, tag="osb", name=f"ob{q_abs}")
                    nc.vector.tensor_scalar_mul(o_sb[:, 0:512], o0, rec)
                    nc.gpsimd.dma_start(
                        out=out_d[q_abs * P:(q_abs + 1) * P, 0:512],
                        in_=o_sb[:, 0:512],
                    )
                    nc.vector.tensor_scalar_mul(o_sb[:, 512:1024], o1, rec)
                    nc.gpsimd.dma_start(
                        out=out_d[q_abs * P:(q_abs + 1) * P, 512:1024],
                        in_=o_sb[:, 512:1024],
                    )

            # ---- orchestration ----
            for si in range(4):
                load_x(si)
            wvh, wvl = load_w8("Wv")
            transpose_group(0)
            for si in range(4, 8):
                load_x(si)
            v_proj_group(0, wvh, wvl)
            transpose_group(1)
            for si in range(8, 12):
                load_x(si)
            v_proj_group(1, wvh, wvl)
            wqh, wql = load_w8("Wq")
            transpose_group(2)
            for si in range(12, 16):
                load_x(si)
            v_proj_group(2, wvh, wvl)
            transpose_group(3)
            v_proj_group(3, wvh, wvl)
            wkh, wkl = load_w8("Wk")
            for sc in range(4):
                qk_proj_sc(sc, wqh, wql, qt8h, qt8l)
            for sc in range(4):
                qk_proj_sc(sc, wkh, wkl, kt8h, kt8l)
            for c in range(4):
                s_stage(2 * c)
                s_stage(2 * c + 1)
                av_stage(c)

    nc.compile()
    return nc


def _get_nc():
    global _built
    if _built is None:
        _built = _build()
    return _built


def _run(inputs, trace=False):
    from concourse.bass_utils import run_bass_kernel_spmd

    x = np.asarray(inputs["x"])
    wq8h, wq8l = _e4m3_hi_lo(inputs["Wq"])
    wk8h, wk8l = _e4m3_hi_lo(inputs["Wk"])
    wv8h, wv8l = _e4m3_hi_lo(inputs["Wv"])
    in_maps = [
        {
            "x16": _bf16_bits(x[c]),
            "Wq8h": wq8h, "Wq8l": wq8l,
            "Wk8h": wk8h, "Wk8l": wk8l,
            "Wv8h": wv8h, "Wv8l": wv8l,
        }
        for c in range(NCORES)
    ]
    res = run_bass_kernel_spmd(
        nc=_get_nc(), in_maps=in_maps, core_ids=list(range(NCORES)), trace=trace
    )
    out = np.stack(
        [
            (
                np.asarray(res.results[c]["out"])
                .view(np.uint16)
                .astype(np.uint32)
                << 16
            ).view(np.float32)
            for c in range(NCORES)
        ],
        axis=0,
    )
    return out, res


def kernel(x, Wq, Wk, Wv):
    out, _ = _run({"x": x, "Wq": Wq, "Wk": Wk, "Wv": Wv}, trace=False)
    return out


# revision 30
# speedup vs baseline: 1.3783x; 1.3783x over previous
"""Causal single-head attention (B=8, S=2048, D=1024, fp32) on 8 NeuronCores.

Data-parallel over batch: one batch element per core, weights replicated.
All matmuls in bf16 (PE weight loads fully pipelined -> ~N*0.417ns/matmul),
fp32 PSUM accumulation, fully SBUF-resident (no DRAM scratch).

Host pre-casts x and W to bf16 (round-to-nearest-even) so the device wire
format is 2-byte: 10MB in / 4MB out per core, no on-device weight casts.

Per-core pipeline:
  1. xT = x.T via PE transposes (bf16, 1 cycle/row)
  2. V  = x @ Wv    -> vsb [ki, kt, e] bf16   (direct SBUF layout)
     QT = Wq.T @ xT -> qt  [e, et, s]  bf16
     KT = Wk.T @ xT -> kt  [e, et, s]  bf16
  3. per 512-wide query chunk c, k-tile k<=4c+3:
       S^T[k,c] accumulated over 8 e-tiles; exp(s/32) on ScalarE -> est bf16
       diagonal tiles multiplied by 0/1 causal mask on GpSimd
  4. per 128-row q tile: rowsum + AV via est-weight-reuse matmul groups
     (rs N=2 self-loading, then 2x N=512 with ldweights=False), normalize
     on VectorE (bf16 out), DMA out. Host widens to fp32.
"""

import numpy as np

B, S, D = 8, 2048, 1024
P = 128
NCORES = 8

_built = None


def _bf16_bits(a):
    """fp32 ndarray -> uint16 bf16 bits, round-to-nearest-even."""
    b = np.ascontiguousarray(a, dtype=np.float32).view(np.uint32)
    r = b + 0x7FFF + ((b >> 16) & 1)
    return (r >> 16).astype(np.uint16)


def _build():
    import concourse.tile as tile
    import concourse.mybir as mybir
    from concourse import bacc

    FP32 = mybir.dt.float32
    BF16 = mybir.dt.bfloat16
    U16 = mybir.dt.uint16
    AF = mybir.ActivationFunctionType

    nc = bacc.Bacc("TRN2", target_bir_lowering=False, debug=False, num_devices=NCORES)
    x_d = nc.dram_tensor("x16", [S, D], U16, kind="ExternalInput").ap().bitcast(BF16)
    wq_d = nc.dram_tensor("Wq16", [D, D], U16, kind="ExternalInput").ap().bitcast(BF16)
    wk_d = nc.dram_tensor("Wk16", [D, D], U16, kind="ExternalInput").ap().bitcast(BF16)
    wv_d = nc.dram_tensor("Wv16", [D, D], U16, kind="ExternalInput").ap().bitcast(BF16)
    out_d = nc.dram_tensor("out", [S, D], U16, kind="ExternalOutput").ap().bitcast(BF16)

    ident_c = nc.inline_tensor(
        (np.eye(P) * 0x3F80).astype(np.uint16), name="ident_c"
    )
    # sliding causal 0/1 mask in bf16 bits: tile j uses cols [(3-j)*128, +512);
    # value at (p, y) = 1 iff y >= p + 384 else 0. cols 896:898 = ones.
    zz = np.arange(896)[None, :]
    pp = np.arange(P)[:, None]
    m01 = np.where(zz >= pp + 384, 0x3F80, 0x0000).astype(np.uint16)
    m01 = np.concatenate([m01, np.full((P, 2), 0x3F80, np.uint16)], axis=1)
    mask_c = nc.inline_tensor(m01, name="mask_c")

    with tile.TileContext(nc) as tc:
        with (
            tc.tile_pool(name="xtp", bufs=1) as xtp,
            tc.tile_pool(name="qtp", bufs=1) as qtp,
            tc.tile_pool(name="ktp", bufs=1) as ktp,
            tc.tile_pool(name="vbp", bufs=1) as vbp,
            tc.tile_pool(name="xsp", bufs=6) as xsp,
            tc.tile_pool(name="wbp", bufs=16) as wbp,
            tc.tile_pool(name="estp", bufs=32) as estp,
            tc.tile_pool(name="osbp", bufs=2) as osbp,
            tc.tile_pool(name="smp", bufs=1) as smp,
            tc.tile_pool(name="rcpp", bufs=2) as rcpp,
            tc.tile_pool(name="ps", bufs=8, space="PSUM") as ps,
        ):
            mask = smp.tile([P, 898], BF16, tag="mask")
            nc.sync.dma_start(out=mask, in_=mask_c.ap().bitcast(BF16))
            ident = smp.tile([P, P], BF16, tag="ident")
            nc.sync.dma_start(out=ident, in_=ident_c.ap().bitcast(BF16))
            ones2 = mask[:, 896:898]
            # memset (no DMA dependency) operand for the p-state warmup
            # matmuls; results never read
            junk = smp.tile([P, 640], BF16, tag="junk")
            nc.vector.memset(junk, 0.0)

            xt = xtp.tile([P, 8, S], BF16, tag="xt")
            qt = qtp.tile([P, 8, S], BF16, tag="qt")
            kt = ktp.tile([P, 8, S], BF16, tag="kt")
            vsb = vbp.tile([P, 16, D], BF16, tag="vsb")

            # PE p-state warmup on uninitialized SBUF (no DMA dependency, so
            # the PE ramps from t~=6.5us while the preamble DMAs stream in).
            for r in range(14):
                dps = ps.tile([P, 512], FP32, tag="ps", name=f"warm{r}")
                nc.tensor.matmul(
                    dps, lhsT=junk[:, 0:P], rhs=junk[:, 128:640],
                    start=True, stop=True,
                )

            x_pend = {}

            def load_x(si):
                x_tile = xsp.tile([P, D], BF16, tag="xs", name=f"x{si}")
                eng = (nc.sync, nc.gpsimd)[si % 2]
                eng.dma_start(out=x_tile, in_=x_d[si * P:(si + 1) * P, :])
                x_pend[si] = x_tile

            def load_w(w_d, wname):
                wb = []
                for kd in range(8):
                    w_t = wbp.tile([P, D], BF16, tag="wb", name=f"{wname}{kd}")
                    eng = (nc.sync, nc.gpsimd)[kd % 2]
                    eng.dma_start(out=w_t, in_=w_d[kd * P:(kd + 1) * P, :])
                    wb.append(w_t)
                return wb

            ncopy = 0

            def copy_cast(out, in_):
                nonlocal ncopy
                eng = (nc.vector.tensor_copy, nc.scalar.copy)[ncopy % 2]
                eng(out=out, in_=in_)
                ncopy += 1

            # ---- phase A: transpose group g (4 s-tiles -> xt columns) ----
            def transpose_group(g):
                xts = [x_pend.pop(si) for si in range(4 * g, 4 * g + 4)]
                for kd in range(8):
                    tp4 = ps.tile([P, 512], BF16, tag="ps", name=f"tp{g}_{kd}")
                    for j in range(4):
                        nc.tensor.matmul(
                            tp4[:, j * P:(j + 1) * P],
                            lhsT=xts[j][:, kd * P:(kd + 1) * P],
                            rhs=ident,
                            is_transpose=True,
                            start=(j == 0),
                            stop=(j == 3),
                        )
                    copy_cast(out=xt[:, kd, g * 512:(g + 1) * 512], in_=tp4)

            # ---- phase B: V projection for group g ----
            def v_proj_group(g, wvb):
                for st_i in range(4 * g, 4 * g + 4):
                    for ec in range(2):
                        pst = ps.tile([P, 512], FP32, tag="ps", name=f"v{st_i}_{ec}")
                        for kd in range(8):
                            nc.tensor.matmul(
                                pst,
                                lhsT=xt[:, kd, st_i * P:(st_i + 1) * P],
                                rhs=wvb[kd][:, ec * 512:(ec + 1) * 512],
                                start=(kd == 0),
                                stop=(kd == 7),
                            )
                        copy_cast(
                            out=vsb[:, st_i, ec * 512:(ec + 1) * 512], in_=pst
                        )

            # ---- phase C/D: Q^T / K^T projection, sc-outer ----
            def qk_proj_sc(sc, wb, dst):
                for et in range(8):
                    pst = ps.tile([P, 512], FP32, tag="ps", name=f"p{sc}_{et}")
                    for kd in range(8):
                        nc.tensor.matmul(
                            pst,
                            lhsT=wb[kd][:, et * P:(et + 1) * P],
                            rhs=xt[:, kd, sc * 512:(sc + 1) * 512],
                            start=(kd == 0),
                            stop=(kd == 7),
                        )
                    copy_cast(out=dst[:, et, sc * 512:(sc + 1) * 512], in_=pst)

            # ---- phase E: S^T + exp for 256-wide chunk c2 ----
            est_tiles = {}

            def s_stage(c2):
                for k in range(2 * c2 + 2):
                    sps = ps.tile([P, 256], FP32, tag="ps", name=f"s{c2}_{k}")
                    for e in range(8):
                        nc.tensor.matmul(
                            sps,
                            lhsT=kt[:, e, k * P:(k + 1) * P],
                            rhs=qt[:, e, c2 * 256:(c2 + 1) * 256],
                            start=(e == 0),
                            stop=(e == 7),
                        )
                    est = estp.tile([P, 256], BF16, tag="est", name=f"e{c2}_{k}")
                    nc.scalar.activation(out=est, in_=sps, func=AF.Exp, scale=0.03125)
                    j2 = k - 2 * c2
                    if j2 >= 0:
                        nc.gpsimd.tensor_mul(
                            est, est, mask[:, (3 - j2) * P:(3 - j2) * P + 256]
                        )
                    est_tiles[(c2, k)] = est

            # ---- phase F: rowsum + AV + normalize for chunk c ----
            def av_stage(c):
                for j in range(4):
                    q_abs = 4 * c + j
                    rs = ps.tile([P, 2], FP32, tag="ps", name=f"rs{q_abs}")
                    o0 = ps.tile([P, 512], FP32, tag="ps", name=f"o0_{q_abs}")
                    o1 = ps.tile([P, 512], FP32, tag="ps", name=f"o1_{q_abs}")
                    c2 = 2 * c + j // 2
                    j2 = j % 2
                    for k in range(q_abs + 1):
                        lhs = est_tiles[(c2, k)][:, j2 * P:(j2 + 1) * P]
                        st = (k == 0)
                        sp = (k == q_abs)
                        nc.tensor.matmul(rs, lhsT=lhs, rhs=ones2, start=st, stop=sp)
                        m2 = nc.tensor.matmul(
                            o0, lhsT=lhs, rhs=vsb[:, k, 0:512], start=st, stop=sp
                        )
                        m2.ins.ldweights = False
                        m3 = nc.tensor.matmul(
                            o1, lhsT=lhs, rhs=vsb[:, k, 512:1024], start=st, stop=sp
                        )
                        m3.ins.ldweights = False
                    rec = rcpp.tile([P, 1], FP32, tag="rcp", name=f"rc{q_abs}")
                    nc.vector.reciprocal(rec, rs[:, 0:1])
                    o_sb = osbp.tile([P, D], BF16, tag="osb", name=f"ob{q_abs}")
                    nc.vector.tensor_scalar_mul(o_sb[:, 0:512], o0, rec)
                    nc.sync.dma_start(
                        out=out_d[q_abs * P:(q_abs + 1) * P, 0:512],
                        in_=o_sb[:, 0:512],
                    )
                    nc.vector.tensor_scalar_mul(o_sb[:, 512:1024], o1, rec)
                    nc.sync.dma_start(
                        out=out_d[q_abs * P:(q_abs + 1) * P, 512:1024],
                        in_=o_sb[:, 512:1024],
                    )

            # ---- orchestration ----
            for si in range(4):
                load_x(si)
            wvb = load_w(wv_d, "wv")
            transpose_group(0)
            for si in range(4, 8):
                load_x(si)
            v_proj_group(0, wvb)
            transpose_group(1)
            for si in range(8, 12):
                load_x(si)
            v_proj_group(1, wvb)
            wqb = load_w(wq_d, "wq")
            transpose_group(2)
            for si in range(12, 16):
                load_x(si)
            v_proj_group(2, wvb)
            transpose_group(3)
            v_proj_group(3, wvb)
            wkb = load_w(wk_d, "wk")
            for sc in range(4):
                qk_proj_sc(sc, wqb, qt)
            for sc in range(4):
                qk_proj_sc(sc, wkb, kt)
            for c in range(4):
                s_stage(2 * c)
                s_stage(2 * c + 1)
                av_stage(c)

    nc.compile()
    return nc


def _get_nc():
    global _built
    if _built is None:
        _built = _build()
    return _built


def _run(inputs, trace=False):
    from concourse.bass_utils import run_bass_kernel_spmd

    x = np.asarray(inputs["x"])
    wq16 = _bf16_bits(inputs["Wq"])
    wk16 = _bf16_bits(inputs["Wk"])
    wv16 = _bf16_bits(inputs["Wv"])
    in_maps = [
        {
            "x16": _bf16_bits(x[c]),
            "Wq16": wq16,
            "Wk16": wk16,
            "Wv16": wv16,
        }
        for c in range(NCORES)
    ]
    res = run_bass_kernel_spmd(
        nc=_get_nc(), in_maps=in_maps, core_ids=list(range(NCORES)), trace=trace
    )
    out = np.stack(
        [
            (
                np.asarray(res.results[c]["out"])
                .view(np.uint16)
                .astype(np.uint32)
                << 16
            ).view(np.float32)
            for c in range(NCORES)
        ],
        axis=0,
    )
    return out, res


def kernel(x, Wq, Wk, Wv):
    out, _ = _run({"x": x, "Wq": Wq, "Wk": Wk, "Wv": Wv}, trace=False)
    return out
